# revision 7
# baseline (speedup 1.0000x reference)
"""Multi-head attention (B=2, S=2048, D=1024, H=16, dk=64) on 8 TRN2 cores.

Sharding: core = (batch, head-group) -> 2 batches x 4 groups of 4 heads.
Each core computes its 4 heads' attention for its batch plus the partial
output projection (row-shard of Wo); the host sums the 4 partials per
batch and adds the output bias.

Per-core kernel layout (feature-major activations):
  qtd_h/ktd_h [128, S]: rows 0-63 and 64-127 both hold head h's
    (q+bq)/8 resp. (k+bk) features -- duplicated so consecutive k-tiles
    of the scores matmul run in opposite PE row-groups (K=64 packing).
  scores^T e = exp(k @ q^T) computed per 128-row k-tile into PSUM groups,
    exp'd by ACT into SBUF.
  PV: attnT[f,q] (+ ones row -> softmax sums) = [v|1]^T @ e, accumulated
    over k-tiles in PSUM; normalized by DVE with a DMA-broadcast recip.
  Out-proj: out[q,d] = attnT^T @ woT chunks.
"""

import math

import numpy as np

import concourse.bass as bass
import concourse.mybir as mybir
import concourse.tile as tile
from concourse import bacc
from concourse.bass_utils import run_bass_kernel_spmd

F32 = mybir.dt.float32
F32R = mybir.dt.float32r
AF = mybir.ActivationFunctionType
ALU = mybir.AluOpType

B = 2
S_FULL = 2048
D_FULL = 1024
H_FULL = 16
DK = 64
HPC = 4  # heads per core
N_CORES = 8


def build_nc(S=S_FULL, D=D_FULL, HPC_=HPC, QB=512, G=2, mm_fast=True):
    """Build the per-core Bass program (SPMD; all cores identical)."""
    KC = D // 128            # d_model chunks
    FEAT = HPC_ * DK         # stacked head features (256)
    HC = FEAT // 128         # feature chunks (2)
    KT = S // 128            # k-token tiles
    NQB = S // QB            # q blocks
    NPQ = QB // 128          # out-proj token tiles per q block
    ND = max(D // 512, 1)    # out-proj n chunks
    NDW = min(512, D)        # out-proj n width
    NG = -(-KT // G)         # score/exp groups per (head, q block)

    MDT = F32R if mm_fast else F32

    def mm(ap):
        return ap

    nc = bacc.Bacc(None, target_bir_lowering=False)
    qt_d = nc.dram_tensor("qt", [D, S], MDT, kind="ExternalInput")
    kt_d = nc.dram_tensor("kt", [D, S], MDT, kind="ExternalInput")
    vtm_d = nc.dram_tensor("vtm", [KT, KC, 128, 128], MDT, kind="ExternalInput")
    wq_d = nc.dram_tensor("wq", [D, FEAT], MDT, kind="ExternalInput")
    wk_d = nc.dram_tensor("wk", [D, FEAT], MDT, kind="ExternalInput")
    wv_d = nc.dram_tensor("wv", [D, FEAT], MDT, kind="ExternalInput")
    bqs_d = nc.dram_tensor("bqs", [128, HC], F32, kind="ExternalInput")
    bk_d = nc.dram_tensor("bk", [128, HC], F32, kind="ExternalInput")
    bv_d = nc.dram_tensor("bv", [FEAT], F32, kind="ExternalInput")
    wo_d = nc.dram_tensor("wo", [FEAT, D], MDT, kind="ExternalInput")
    out_d = nc.dram_tensor("out", [S, D], F32, kind="ExternalOutput")

    with tile.TileContext(nc) as tc:
        with (
            tc.tile_pool(name="wpool", bufs=1) as wpool,
            tc.tile_pool(name="persist", bufs=1) as persist,
            tc.tile_pool(name="epool", bufs=6) as epool,
            tc.tile_pool(name="apool", bufs=2) as apool,
            tc.tile_pool(name="rpool", bufs=2) as rpool,
            tc.tile_pool(name="opool", bufs=2) as opool,
        ):
            # ---- weights and biases ----
            wq_sb = wpool.tile([128, KC, FEAT], MDT, tag="wq")
            wk_sb = wpool.tile([128, KC, FEAT], MDT, tag="wk")
            wv_sb = wpool.tile([128, KC, FEAT], MDT, tag="wv")
            for c in range(KC):
                nc.sync.dma_start(out=wq_sb[:, c, :], in_=wq_d[c * 128:(c + 1) * 128, :])
                nc.sync.dma_start(out=wk_sb[:, c, :], in_=wk_d[c * 128:(c + 1) * 128, :])
                nc.sync.dma_start(out=wv_sb[:, c, :], in_=wv_d[c * 128:(c + 1) * 128, :])
            wo_sb = wpool.tile([128, HC, D], MDT, tag="wo")
            for c in range(HC):
                nc.sync.dma_start(out=wo_sb[:, c, :], in_=wo_d[c * 128:(c + 1) * 128, :])
            bqs_sb = wpool.tile([128, HC], F32, tag="bqs")
            nc.sync.dma_start(out=bqs_sb[:], in_=bqs_d[:])
            bk_sb = wpool.tile([128, HC], F32, tag="bk")
            nc.sync.dma_start(out=bk_sb[:], in_=bk_d[:])
            bv_sb = wpool.tile([128, FEAT], F32, tag="bv")
            bv_ap = bv_d[:]
            bv_bcast = bass.AP(
                tensor=bv_ap.tensor, offset=bv_ap.offset,
                ap=[[0, 128]] + [list(d) for d in bv_ap.ap],
            )
            nc.sync.dma_start(out=bv_sb[:], in_=bv_bcast)

            # persistent per-head duplicated activations
            qtd = [persist.tile([128, S], MDT, tag=f"qtd{h}", name=f"qtd{h}") for h in range(HPC_)]
            ktd = [persist.tile([128, S], MDT, tag=f"ktd{h}", name=f"ktd{h}") for h in range(HPC_)]
            va = [persist.tile([128, HPC_, DK + 1], MDT, tag=f"va{m}", name=f"va{m}") for m in range(KT)]

            with (
                tc.tile_pool(name="psA", bufs=NQB * HC, space="PSUM") as psA,
                tc.tile_pool(name="stream", bufs=3) as stream,
                tc.tile_pool(name="vstream", bufs=6) as vstream,
            ):
                # ---- q/k projections (feature-major, duplicated) ----
                for xt_d, w_sb, dst, bias_sb, scale in (
                    (qt_d, wq_sb, qtd, bqs_sb, 0.125),
                    (kt_d, wk_sb, ktd, bk_sb, None),
                ):
                    psums = {}
                    for mc in range(HC):
                        for qb in range(NQB):
                            psums[mc, qb] = psA.tile([128, QB], F32, tag="proj", name="projps")
                    for c in range(KC):
                        xt_sb = stream.tile([128, S], MDT, tag="stream")
                        nc.sync.dma_start(out=xt_sb[:], in_=xt_d[c * 128:(c + 1) * 128, :])
                        for mc in range(HC):
                            for qb in range(NQB):
                                nc.tensor.matmul(
                                    psums[mc, qb][:],
                                    mm(w_sb[:, c, mc * 128:(mc + 1) * 128]),
                                    mm(xt_sb[:, qb * QB:(qb + 1) * QB]),
                                    start=(c == 0), stop=(c == KC - 1),
                                )
                    for mc in range(HC):
                        for qb in range(NQB):
                            ps = psums[mc, qb]
                            for hh in range(2):
                                h = mc * 2 + hh
                                src = ps[hh * 64:(hh + 1) * 64, :]
                                b_ap = bias_sb[hh * 64:(hh + 1) * 64, mc:mc + 1]
                                for half in range(2):
                                    dst_ap = dst[h][half * 64:(half + 1) * 64,
                                                    qb * QB:(qb + 1) * QB]
                                    if scale is not None:
                                        nc.vector.tensor_scalar(
                                            dst_ap, src, scale, b_ap,
                                            ALU.mult, ALU.add,
                                        )
                                    else:
                                        nc.vector.tensor_scalar(
                                            dst_ap, src, b_ap, None, ALU.add,
                                        )

                # ---- v projection (token-major, augmented with ones col) ----
                bv_view = bv_sb[:].rearrange("p (h d) -> p h d", h=HPC_)
                for m in range(KT):
                    ps = psA.tile([128, FEAT], F32, tag="proj")
                    for c in range(KC):
                        vt_sb = vstream.tile([128, 128], MDT, tag="vs")
                        nc.sync.dma_start(out=vt_sb[:], in_=vtm_d[m, c, :, :])
                        nc.tensor.matmul(
                            ps[:], mm(vt_sb[:]), mm(wv_sb[:, c, :]),
                            start=(c == 0), stop=(c == KC - 1),
                        )
                    vam = va[m]
                    nc.vector.tensor_scalar(
                        vam[:, :, DK:DK + 1], bv_sb[:, 0:HPC_].unsqueeze(2),
                        0.0, 1.0, ALU.mult, ALU.add,
                    )
                    ps_view = ps[:].rearrange("p (h d) -> p h d", h=HPC_)
                    nc.vector.tensor_tensor(
                        out=vam[:, :, 0:DK], in0=ps_view, in1=bv_view, op=ALU.add,
                    )

            # ---- attention + output projection ----
            with tc.tile_pool(name="psB", bufs=2, space="PSUM") as psB:
                for qb in range(NQB):
                    at = [apool.tile([128, QB], MDT, tag=f"at{c}", name=f"at{c}") for c in range(HC)]
                    for h in range(HPC_):
                        pv = psB.tile([128, QB], F32, tag="pv")
                        for g in range(NG):
                            kts = list(range(g * G, min((g + 1) * G, KT)))
                            sp = psB.tile([128, G, QB], F32, tag="spp")
                            for j, kt in enumerate(kts):
                                b0 = (kt % 2) * 64
                                nc.tensor.matmul(
                                    sp[:, j, :],
                                    mm(ktd[h][b0:b0 + 64, kt * 128:(kt + 1) * 128]),
                                    mm(qtd[h][b0:b0 + 64, qb * QB:(qb + 1) * QB]),
                                    start=True, stop=True,
                                )
                            et = epool.tile([128, G, QB], MDT, tag="et")
                            L = len(kts)
                            nc.scalar.activation(
                                out=et[:, 0:L, :], in_=sp[:, 0:L, :], func=AF.Exp,
                            )
                            for j, kt in enumerate(kts):
                                nc.tensor.matmul(
                                    pv[0:DK + 1, :],
                                    mm(va[kt][:, h, :]),
                                    mm(et[:, j, :]),
                                    start=(kt == 0), stop=(kt == KT - 1),
                                )
                        rt = rpool.tile([64, QB], F32, tag="rt")
                        nc.vector.reciprocal(out=rt[0:1, :], in_=pv[DK:DK + 1, :])
                        rb = rpool.tile([64, QB], F32, tag="rb")
                        rt_row = rt[0:1, :]
                        # replicate the single recip row to 64 partitions:
                        # source iterates the row 64x via a zero-step free dim
                        rt_bcast = bass.AP(
                            tensor=rt_row.tensor, offset=rt_row.offset,
                            ap=[list(rt_row.ap[0]), [0, 64]]
                            + [list(d) for d in rt_row.ap[1:]],
                        )
                        nc.sync.dma_start(out=rb[:], in_=rt_bcast)
                        r0 = (h % 2) * 64
                        nc.vector.tensor_tensor(
                            out=at[h // 2][r0:r0 + 64, :],
                            in0=pv[0:DK, :], in1=rb[:], op=ALU.mult,
                        )
                    for mp in range(NPQ):
                        out_t = opool.tile([128, D], F32, tag="ot")
                        for n in range(ND):
                            op = psB.tile([128, NDW], F32, tag="op")
                            for c in range(HC):
                                nc.tensor.matmul(
                                    op[:],
                                    mm(at[c][:, mp * 128:(mp + 1) * 128]),
                                    mm(wo_sb[:, c, n * NDW:(n + 1) * NDW]),
                                    start=(c == 0), stop=(c == HC - 1),
                                )
                            nc.vector.tensor_copy(
                                out=out_t[:, n * NDW:(n + 1) * NDW], in_=op[:],
                            )
                        row = qb * QB + mp * 128
                        nc.sync.dma_start(out=out_d[row:row + 128, :], in_=out_t[:])

    nc.finalize()
    return nc


def make_in_map(Qb, Kb, Vb, Wq4, bq4, Wk4, bk4, Wv4, bv4, Wo4):
    """Host-side shard prep for one core. Qb/Kb/Vb: [S, D] fp32 for this
    batch; *4 weights: this core's 4 heads ([4, D, DK] / [4, DK]); Wo4:
    [D, 4*DK] columns of Wo for these heads."""
    S, D = Qb.shape
    KT = S // 128
    KC = D // 128
    f32 = np.float32
    qt = np.ascontiguousarray(Qb.T, dtype=f32)
    kt = np.ascontiguousarray(Kb.T, dtype=f32)
    # vtm[m, c, i, j] = V[m*128+j, c*128+i]
    vtm = np.ascontiguousarray(
        Vb.reshape(KT, 128, KC, 128).transpose(0, 2, 3, 1), dtype=f32)
    wq = np.ascontiguousarray(Wq4.transpose(1, 0, 2).reshape(D, -1), dtype=f32)
    wk = np.ascontiguousarray(Wk4.transpose(1, 0, 2).reshape(D, -1), dtype=f32)
    wv = np.ascontiguousarray(Wv4.transpose(1, 0, 2).reshape(D, -1), dtype=f32)
    bqs = np.ascontiguousarray(
        (bq4.reshape(-1) * 0.125).reshape(-1, 128).T, dtype=f32)
    bk = np.ascontiguousarray(bk4.reshape(-1).reshape(-1, 128).T, dtype=f32)
    bv = np.ascontiguousarray(bv4.reshape(-1), dtype=f32)
    wo = np.ascontiguousarray(Wo4.T, dtype=f32)
    return dict(qt=qt, kt=kt, vtm=vtm, wq=wq, wk=wk, wv=wv,
                bqs=bqs, bk=bk, bv=bv, wo=wo)


_NC = None


def _get_nc():
    global _NC
    if _NC is None:
        _NC = build_nc()
    return _NC


def kernel(Q, K, V, Wq, bq, Wk, bk, Wv, bv, Wo, bo, _collect=None):
    Q = np.asarray(Q, np.float32)
    K = np.asarray(K, np.float32)
    V = np.asarray(V, np.float32)
    Wq = np.asarray(Wq, np.float32)
    bq = np.asarray(bq, np.float32)
    Wk = np.asarray(Wk, np.float32)
    bk = np.asarray(bk, np.float32)
    Wv = np.asarray(Wv, np.float32)
    bv = np.asarray(bv, np.float32)
    Wo = np.asarray(Wo, np.float32)
    bo = np.asarray(bo, np.float32)

    nc = _get_nc()
    in_maps = []
    for b in range(B):
        for g in range(N_CORES // B):
            hs = slice(g * HPC, (g + 1) * HPC)
            cols = slice(g * HPC * DK, (g + 1) * HPC * DK)
            in_maps.append(make_in_map(
                Q[b], K[b], V[b], Wq[hs], bq[hs], Wk[hs], bk[hs],
                Wv[hs], bv[hs], Wo[:, cols]))

    kwargs = _collect.get("kwargs", {}) if _collect else {}
    res = run_bass_kernel_spmd(nc, in_maps, core_ids=list(range(N_CORES)), **kwargs)
    if _collect is not None:
        _collect["res"] = res

    out = np.zeros((B, S_FULL, D_FULL), np.float32)
    gpb = N_CORES // B
    for b in range(B):
        acc = res.results[b * gpb]["out"].astype(np.float32).copy()
        for g in range(1, gpb):
            acc += res.results[b * gpb + g]["out"]
        out[b] = acc + bo[None, :]
    return out


# revision 10
# speedup vs baseline: 1.3087x; 1.3087x over previous
"""Multi-head attention (B=2, S=2048, D=1024, H=16, dk=64) on 8 TRN2 cores.

Sharding: core = (batch, head-group) -> 2 batches x 4 groups of 4 heads.
Each core computes its 4 heads' attention for its batch plus the partial
output projection (row-shard of Wo); the host sums the 4 partials per
batch and adds the output bias.

Per-core kernel (activations feature-major, matmul operands bf16):
  qtd_h/ktd_h [128, S]: rows 0-63 and 64-127 both hold head h's
    (q+bq)/8 resp. (k+bk) features -- duplicated so consecutive k-tiles
    of the scores matmul run in opposite PE row-groups (K=64 packing).
  scores^T: e = exp(k @ q^T) per 128-row k-tile into PSUM groups,
    exp'd by ACT into SBUF (bf16).
  PV: attnT[f,q] (+ ones row -> softmax sums) = [v|1]^T @ e, accumulated
    over k-tiles in PSUM; normalized with a DMA-broadcast reciprocal.
  Out-proj: out[q,d] = attnT^T @ woT chunks, emitted one q-block late so
    the PE fills the ACT-bound window of the next block's first head.
One PSUM pool spans the whole kernel (no phase barrier): tags
  spp [128,G,512] x2 + pv [128,512] x4 = 8 banks.
"""

import numpy as np
import ml_dtypes

import concourse.bass as bass
import concourse.mybir as mybir
import concourse.tile as tile
from concourse import bacc
from concourse.bass_utils import run_bass_kernel_spmd

F32 = mybir.dt.float32
BF16 = mybir.dt.bfloat16
AF = mybir.ActivationFunctionType
ALU = mybir.AluOpType

B = 2
S_FULL = 2048
D_FULL = 1024
H_FULL = 16
DK = 64
HPC = 4  # heads per core
N_CORES = 8
BF16_NP = ml_dtypes.bfloat16


def build_nc(S=S_FULL, D=D_FULL, HPC_=HPC, QB=512, G=2, mm_fast=True):
    """Build the per-core Bass program (SPMD; all cores identical)."""
    KC = D // 128            # d_model chunks
    FEAT = HPC_ * DK         # stacked head features (256)
    HC = FEAT // 128         # feature chunks (2)
    KT = S // 128            # k-token tiles
    NQB = S // QB            # q blocks
    NPQ = QB // 128          # out-proj token tiles per q block
    ND = max(D // 512, 1)    # out-proj n chunks
    NDW = min(512, D)        # out-proj n width
    NG = -(-KT // G)         # score/exp groups per (head, q block)

    MDT = BF16 if mm_fast else F32

    nc = bacc.Bacc(None, target_bir_lowering=False)
    qt_d = nc.dram_tensor("qt", [D, S], MDT, kind="ExternalInput")
    kt_d = nc.dram_tensor("kt", [D, S], MDT, kind="ExternalInput")
    vtm_d = nc.dram_tensor("vtm", [KT, 128, KC * 128], MDT, kind="ExternalInput")
    wq_d = nc.dram_tensor("wq", [D, FEAT], MDT, kind="ExternalInput")
    wk_d = nc.dram_tensor("wk", [D, FEAT], MDT, kind="ExternalInput")
    wv_d = nc.dram_tensor("wv", [D, FEAT], MDT, kind="ExternalInput")
    bqs_d = nc.dram_tensor("bqs", [128, HC], F32, kind="ExternalInput")
    bk_d = nc.dram_tensor("bk", [128, HC], F32, kind="ExternalInput")
    bv_d = nc.dram_tensor("bv", [FEAT], F32, kind="ExternalInput")
    wo_d = nc.dram_tensor("wo", [FEAT, D], MDT, kind="ExternalInput")
    out_d = nc.dram_tensor("out", [S, D], F32, kind="ExternalOutput")

    with tile.TileContext(nc) as tc:
        with (
            tc.tile_pool(name="wpool", bufs=1) as wpool,
            tc.tile_pool(name="persist", bufs=1) as persist,
            tc.tile_pool(name="stream", bufs=KC) as stream,
            tc.tile_pool(name="vstream", bufs=3) as vstream,
            tc.tile_pool(name="epool", bufs=6) as epool,
            tc.tile_pool(name="apool", bufs=2) as apool,
            tc.tile_pool(name="rpool", bufs=2) as rpool,
            tc.tile_pool(name="opool", bufs=3) as opool,
            tc.tile_pool(name="psum", bufs=2, space="PSUM") as psum,
        ):
            # ---- weights and biases ----
            wq_sb = wpool.tile([128, KC, FEAT], MDT, tag="wq")
            wk_sb = wpool.tile([128, KC, FEAT], MDT, tag="wk")
            wv_sb = wpool.tile([128, KC, FEAT], MDT, tag="wv")
            for c in range(KC):
                nc.sync.dma_start(out=wk_sb[:, c, :], in_=wk_d[c * 128:(c + 1) * 128, :])
                nc.sync.dma_start(out=wq_sb[:, c, :], in_=wq_d[c * 128:(c + 1) * 128, :])
                nc.sync.dma_start(out=wv_sb[:, c, :], in_=wv_d[c * 128:(c + 1) * 128, :])
            wo_sb = wpool.tile([128, HC, D], MDT, tag="wo")
            for c in range(HC):
                nc.sync.dma_start(out=wo_sb[:, c, :], in_=wo_d[c * 128:(c + 1) * 128, :])
            bqs_sb = wpool.tile([128, HC], F32, tag="bqs")
            nc.sync.dma_start(out=bqs_sb[:], in_=bqs_d[:])
            bk_sb = wpool.tile([128, HC], F32, tag="bk")
            nc.sync.dma_start(out=bk_sb[:], in_=bk_d[:])
            bv_sb = wpool.tile([128, FEAT], F32, tag="bv")
            bv_ap = bv_d[:]
            bv_bcast = bass.AP(
                tensor=bv_ap.tensor, offset=bv_ap.offset,
                ap=[[0, 128]] + [list(d) for d in bv_ap.ap],
            )
            nc.sync.dma_start(out=bv_sb[:], in_=bv_bcast)

            # persistent per-head duplicated activations
            qtd = [persist.tile([128, S], MDT, tag=f"qtd{h}", name=f"qtd{h}")
                   for h in range(HPC_)]
            ktd = [persist.tile([128, S], MDT, tag=f"ktd{h}", name=f"ktd{h}")
                   for h in range(HPC_)]
            va = [persist.tile([128, HPC_, DK + 1], MDT, tag=f"va{m}", name=f"va{m}")
                  for m in range(KT)]

            # ---- k/q projections (feature-major, duplicated outputs) ----
            # all d-chunks of the source stay resident (bf16); per q-block
            # only 2 psums (feature chunks) are live -> shared pv tag.
            for xt_d, w_sb, dst, bias_sb, scale in (
                (kt_d, wk_sb, ktd, bk_sb, None),
                (qt_d, wq_sb, qtd, bqs_sb, 0.125),
            ):
                xts = []
                for c in range(KC):
                    xt_sb = stream.tile([128, S], MDT, tag="stream", name="xt")
                    nc.sync.dma_start(out=xt_sb[:], in_=xt_d[c * 128:(c + 1) * 128, :])
                    xts.append(xt_sb)
                for qb in range(NQB):
                    pss = [psum.tile([128, QB], F32, tag="pv", name="projps", bufs=4)
                           for _ in range(HC)]
                    for c in range(KC):
                        for mc in range(HC):
                            nc.tensor.matmul(
                                pss[mc][:],
                                w_sb[:, c, mc * 128:(mc + 1) * 128],
                                xts[c][:, qb * QB:(qb + 1) * QB],
                                start=(c == 0), stop=(c == KC - 1),
                            )
                    for mc in range(HC):
                        ps = pss[mc]
                        for hh in range(2):
                            h = mc * 2 + hh
                            src = ps[hh * 64:(hh + 1) * 64, :]
                            b_ap = bias_sb[hh * 64:(hh + 1) * 64, mc:mc + 1]
                            for half in range(2):
                                dst_ap = dst[h][half * 64:(half + 1) * 64,
                                                qb * QB:(qb + 1) * QB]
                                if half == 0:
                                    # DVE path
                                    if scale is not None:
                                        nc.vector.tensor_scalar(
                                            dst_ap, src, scale, b_ap,
                                            ALU.mult, ALU.add,
                                        )
                                    else:
                                        nc.vector.tensor_scalar(
                                            dst_ap, src, b_ap, None, ALU.add,
                                        )
                                else:
                                    # ACT path (idle until attention starts)
                                    nc.scalar.activation(
                                        out=dst_ap, in_=src, func=AF.Identity,
                                        bias=b_ap,
                                        scale=scale if scale is not None else 1.0,
                                    )

            # ---- v projection (token-major, augmented with ones col) ----
            bv_view = bv_sb[:].rearrange("p (h d) -> p h d", h=HPC_)
            for m in range(KT):
                vt_sb = vstream.tile([128, KC, 128], MDT, tag="vs", name="vt")
                nc.sync.dma_start(out=vt_sb[:], in_=vtm_d[m, :, :])
                ps = psum.tile([128, FEAT], F32, tag="pv", name="projv", bufs=4)
                for c in range(KC):
                    nc.tensor.matmul(
                        ps[:], vt_sb[:, c, :], wv_sb[:, c, :],
                        start=(c == 0), stop=(c == KC - 1),
                    )
                vam = va[m]
                nc.vector.tensor_scalar(
                    vam[:, :, DK:DK + 1], bv_sb[:, 0:HPC_].unsqueeze(2),
                    0.0, 1.0, ALU.mult, ALU.add,
                )
                ps_view = ps[:].rearrange("p (h d) -> p h d", h=HPC_)
                nc.vector.tensor_tensor(
                    out=vam[:, :, 0:DK], in0=ps_view, in1=bv_view, op=ALU.add,
                )

            # ---- attention + (deferred, spread) output projection ----
            pending = []  # (at, qb, mp) out-proj units

            def emit_outproj_unit():
                at_p, qb_p, mp = pending.pop(0)
                out_t = opool.tile([128, D], F32, tag="ot", name="ot")
                for n in range(ND):
                    op = psum.tile([128, NDW], F32, tag="pv", name="opps", bufs=4)
                    for c in range(HC):
                        nc.tensor.matmul(
                            op[:],
                            at_p[c][:, mp * 128:(mp + 1) * 128],
                            wo_sb[:, c, n * NDW:(n + 1) * NDW],
                            start=(c == 0), stop=(c == HC - 1),
                        )
                    nc.vector.tensor_copy(
                        out=out_t[:, n * NDW:(n + 1) * NDW], in_=op[:],
                    )
                row = qb_p * QB + mp * 128
                nc.sync.dma_start(out=out_d[row:row + 128, :], in_=out_t[:])

            for qb in range(NQB):
                at = [apool.tile([128, QB], MDT, tag=f"at{c}", name=f"at{c}")
                      for c in range(HC)]
                for h in range(HPC_):
                    pv = psum.tile([128, QB], F32, tag="pv", name="pvps", bufs=4)
                    for g in range(NG):
                        kts = list(range(g * G, min((g + 1) * G, KT)))
                        sp = psum.tile([128, G, QB], F32, tag="spp", name="spps")
                        for j, kt in enumerate(kts):
                            b0 = (kt % 2) * 64
                            nc.tensor.matmul(
                                sp[:, j, :],
                                ktd[h][b0:b0 + 64, kt * 128:(kt + 1) * 128],
                                qtd[h][b0:b0 + 64, qb * QB:(qb + 1) * QB],
                                start=True, stop=True,
                            )
                        et = epool.tile([128, G, QB], MDT, tag="et", name="et")
                        L = len(kts)
                        nc.scalar.activation(
                            out=et[:, 0:L, :], in_=sp[:, 0:L, :], func=AF.Exp,
                        )
                        for j, kt in enumerate(kts):
                            nc.tensor.matmul(
                                pv[0:DK + 1, :],
                                va[kt][:, h, :],
                                et[:, j, :],
                                start=(kt == 0), stop=(kt == KT - 1),
                            )
                        if h == 0 and g >= 1 and pending:
                            emit_outproj_unit()
                    if h == 1 and pending:
                        while pending:
                            emit_outproj_unit()
                    rt = rpool.tile([64, QB], F32, tag="rt", name="rt")
                    nc.vector.reciprocal(out=rt[0:1, :], in_=pv[DK:DK + 1, :])
                    rb = rpool.tile([64, QB], F32, tag="rb", name="rb")
                    rt_row = rt[0:1, :]
                    # replicate the recip row to 64 partitions: source
                    # iterates the row 64x via a zero-step free dim
                    rt_bcast = bass.AP(
                        tensor=rt_row.tensor, offset=rt_row.offset,
                        ap=[list(rt_row.ap[0]), [0, 64]]
                        + [list(d) for d in rt_row.ap[1:]],
                    )
                    nc.sync.dma_start(out=rb[:], in_=rt_bcast)
                    r0 = (h % 2) * 64
                    nc.vector.tensor_tensor(
                        out=at[h // 2][r0:r0 + 64, :],
                        in0=pv[0:DK, :], in1=rb[:], op=ALU.mult,
                    )
                for mp in range(NPQ):
                    pending.append((at, qb, mp))
            while pending:
                emit_outproj_unit()

    nc.finalize()
    return nc


def make_in_map(Qb, Kb, Vb, Wq4, bq4, Wk4, bk4, Wv4, bv4, Wo4, mm_fast=True):
    """Host-side shard prep for one core. Qb/Kb/Vb: [S, D] fp32 for this
    batch; *4 weights: this core's 4 heads ([4, D, DK] / [4, DK]); Wo4:
    [D, 4*DK] columns of Wo for these heads."""
    S, D = Qb.shape
    KT = S // 128
    KC = D // 128
    f32 = np.float32
    mdt = BF16_NP if mm_fast else f32
    qt = np.ascontiguousarray(Qb.T).astype(mdt)
    kt = np.ascontiguousarray(Kb.T).astype(mdt)
    # vtm[m, i, c*128+j] = V[m*128+j, c*128+i]
    vtm = np.ascontiguousarray(
        Vb.reshape(KT, 128, KC, 128).transpose(0, 3, 2, 1).reshape(KT, 128, KC * 128)
    ).astype(mdt)
    wq = np.ascontiguousarray(Wq4.transpose(1, 0, 2).reshape(D, -1)).astype(mdt)
    wk = np.ascontiguousarray(Wk4.transpose(1, 0, 2).reshape(D, -1)).astype(mdt)
    wv = np.ascontiguousarray(Wv4.transpose(1, 0, 2).reshape(D, -1)).astype(mdt)
    bqs = np.ascontiguousarray(
        (bq4.reshape(-1) * 0.125).reshape(-1, 128).T, dtype=f32)
    bk = np.ascontiguousarray(bk4.reshape(-1).reshape(-1, 128).T, dtype=f32)
    bv = np.ascontiguousarray(bv4.reshape(-1), dtype=f32)
    wo = np.ascontiguousarray(Wo4.T).astype(mdt)
    return dict(qt=qt, kt=kt, vtm=vtm, wq=wq, wk=wk, wv=wv,
                bqs=bqs, bk=bk, bv=bv, wo=wo)


_NC = None


def _get_nc():
    global _NC
    if _NC is None:
        _NC = build_nc()
    return _NC


def kernel(Q, K, V, Wq, bq, Wk, bk, Wv, bv, Wo, bo, _collect=None):
    Q = np.asarray(Q, np.float32)
    K = np.asarray(K, np.float32)
    V = np.asarray(V, np.float32)
    Wq = np.asarray(Wq, np.float32)
    bq = np.asarray(bq, np.float32)
    Wk = np.asarray(Wk, np.float32)
    bk = np.asarray(bk, np.float32)
    Wv = np.asarray(Wv, np.float32)
    bv = np.asarray(bv, np.float32)
    Wo = np.asarray(Wo, np.float32)
    bo = np.asarray(bo, np.float32)

    nc = _get_nc()
    in_maps = []
    for b in range(B):
        for g in range(N_CORES // B):
            hs = slice(g * HPC, (g + 1) * HPC)
            cols = slice(g * HPC * DK, (g + 1) * HPC * DK)
            in_maps.append(make_in_map(
                Q[b], K[b], V[b], Wq[hs], bq[hs], Wk[hs], bk[hs],
                Wv[hs], bv[hs], Wo[:, cols]))

    kwargs = _collect.get("kwargs", {}) if _collect else {}
    res = run_bass_kernel_spmd(nc, in_maps, core_ids=list(range(N_CORES)), **kwargs)
    if _collect is not None:
        _collect["res"] = res

    out = np.zeros((B, S_FULL, D_FULL), np.float32)
    gpb = N_CORES // B
    for b in range(B):
        acc = res.results[b * gpb]["out"].astype(np.float32).copy()
        for g in range(1, gpb):
            acc += res.results[b * gpb + g]["out"]
        out[b] = acc + bo[None, :]
    return out
